# revision 9
# baseline (speedup 1.0000x reference)
"""Multi-head attention layer on 8 Trainium2 NeuronCores.

Problem: B=4, S=2048, D=1024, H=16 heads (DH=64), int mask over keys.
Sharding: core c -> batch b=c//2, head-group hg=c%2 (8 heads each).
Each core computes its heads' full S x S attention independently; no
collectives.  Host-side prep folds everything expensive into the input
layout:

  - X is passed TRANSPOSED ([D, S], d on partitions) so Q^T/K^T come out
    of the projection matmuls directly in the [d, S] layout the attention
    matmuls consume (zero on-device transposes).
  - Wk is pre-scaled by 1/sqrt(DH) on the host.
  - Masked-out keys are compacted away on the host; PAD keys (tail of the
    last tiles) are killed by ZEROING their ones-columns in the packed V
    (a DMA'd 0/1 block) -- no mask matmuls at all: a pad key contributes
    exp(0)*0 = 0 to both the numerator and the denominator.
  - V gets 64 ones-columns per head so the PV matmul emits the numerator on
    partitions 0-63 AND the softmax denominator replicated across 64-127.

The kernel is ONE software-pipelined instruction stream.  Scores run in
the PE's 64-row tiling mode (head pairs packed onto array halves T0/T8
run concurrently -- a pair of N=512 score matmuls takes ~216 ns, not
2x216).  Because switching the PE between 64-row and 128-row tiling
drains the array (~106 ns), the stream is emitted in TWO-TICK windows:
[scores g, scores g+1] in 64-row mode, then [PV g-LAG, PV g-LAG+1 and
projection chunks] in 128-row mode, paying 2 mode switches per 2 ticks
instead of per tick.  QKV projection chunks are interleaved into the
128-mode sections just-in-time.
"""

import os
import sys

import numpy as np
import ml_dtypes

for _p in ("/opt/trn_rl_repo", "/opt/pypackages"):
    if os.path.isdir(_p) and _p not in sys.path:
        sys.path.append(_p)

import concourse.bass as bass
import concourse.mybir as mybir
import concourse.tile as tile
from concourse.tile import add_dep_helper
from contextlib import ExitStack

BF16 = mybir.dt.bfloat16
F32 = mybir.dt.float32

B, S, D, H, DH = 4, 2048, 1024, 16, 64
NCORES = 8
DCOL = 512          # head-group width (8 heads x 64)
NM = 4              # 128-wide dcol tiles of the head group
NQC = 4             # 512-wide query chunks
EXPFN = mybir.ActivationFunctionType.Exp


def _chunks(total, size=512):
    out, o = [], 0
    while o < total:
        c = min(size, total - o)
        out.append(slice(o, o + c))
        o += c
    return out


def build_nc(nk: int, skv: int, npt: int) -> bass.Bass:
    """nk: K-tiles over the hidden dim (8, or 9 with a bias row).
    skv: compacted+padded key/value sequence length (multiple of 128).
    npt: number of tail key tiles that may contain pad keys; their
    ones-columns come from the DMA'd `vones` 0/1 block."""
    NK = nk
    KPAD = NK * 128
    NJ = skv // 128     # key tiles for scores / PV / V-projection
    WIN = NQC * NJ      # attention ticks per head-pair window
    nc = bass.Bass()
    xt_d = nc.declare_dram_parameter("xt", [KPAD, S], BF16, isOutput=False)
    xkv_d = nc.declare_dram_parameter("xkv", [KPAD, skv], BF16,
                                      isOutput=False)
    # wq/wk are packed m-major on the host ([(m k p), 128]) so a single
    # m-block (one head pair's 128 columns) is ONE contiguous DMA.
    wq_d = nc.declare_dram_parameter("wq", [NM * KPAD, 128], BF16,
                                     isOutput=False)
    wk_d = nc.declare_dram_parameter("wk", [NM * KPAD, 128], BF16,
                                     isOutput=False)
    wv_d = nc.declare_dram_parameter("wv", [KPAD, DCOL], BF16, isOutput=False)
    # 0/1 ones-column block for the last npt key tiles (kills pad keys)
    vo_d = nc.declare_dram_parameter("vones", [128, npt * 512], BF16,
                                     isOutput=False)
    # Output is stored q-chunk-major ([q, dcol-row, 512]) so each [64, 512]
    # store block is one CONTIGUOUS 128 KB DMA (the [dcol, S] layout's
    # 2 KB-strided rows cost many small descriptors); host re-assembles.
    out_d = nc.declare_dram_parameter("out", [NQC * DCOL, 512], F32,
                                      isOutput=True)
    # DRAM bounce rows for the softmax denominators: the 512 reciprocals of
    # one (it, head) unit are computed on a [128, 4] layout (cheap per-lane
    # DVE reciprocal), bounced through DRAM, and read back broadcast to 64
    # partitions with a stride-0 source AP (DRAM APs allow it; SBUF ones
    # don't, and a stride-0 SBUF source wedges the DMA engine).
    dbn_d = nc.declare_dram_parameter("dbn", [2 * NM * NQC, 512], F32,
                                      isOutput=True)

    with tile.TileContext(nc) as tc, ExitStack() as ctx:
        const = ctx.enter_context(tc.tile_pool(name="const", bufs=1))
        spool = ctx.enter_context(tc.tile_pool(name="sc", bufs=2, space="PSUM"))
        ppool = ctx.enter_context(tc.tile_pool(name="pj", bufs=2, space="PSUM"))
        pvpool = ctx.enter_context(tc.tile_pool(name="pv", bufs=1, space="PSUM"))
        expool = ctx.enter_context(tc.tile_pool(name="ex", bufs=13))
        comb = ctx.enter_context(tc.tile_pool(name="comb", bufs=6))
        lastp = ctx.enter_context(tc.tile_pool(name="lastp", bufs=1))
        outp = ctx.enter_context(tc.tile_pool(name="outp", bufs=5))

        # ---------------- persistent SBUF tensors ----------------
        # DMA-written tensors are SPLIT into one tile per transfer: Tile's
        # RAW deps for DMA writes are tile-granular, so a single big tile
        # written by several DMAs would gate every reader on the LAST
        # transfer (measured: first matmul at 17.7 us instead of ~6).
        q_slices = _chunks(S)
        kv_slices = _chunks(skv)
        xts = [const.tile([128, NK, s.stop - s.start], BF16, tag=f"xt{i}",
                          name=f"xt{i}") for i, s in enumerate(q_slices)]
        xkvs = [const.tile([128, NK, s.stop - s.start], BF16, tag=f"xkv{i}",
                           name=f"xkv{i}") for i, s in enumerate(kv_slices)]
        wq0 = const.tile([128, NK, 128], BF16, tag="wq0", name="wq0")
        wqr = const.tile([128, NM - 1, NK, 128], BF16, tag="wqr", name="wqr")
        wk0 = const.tile([128, NK, 128], BF16, tag="wk0", name="wk0")
        wkr = const.tile([128, NM - 1, NK, 128], BF16, tag="wkr", name="wkr")
        wv = const.tile([128, NK, DCOL], BF16, tag="wv", name="wv")
        qt = const.tile([128, NM, S], BF16, tag="qt")    # Q^T  [dcol, S]
        kt = const.tile([128, NM, skv], BF16, tag="kt")  # K^T (pre-scaled)
        # V (cols 0-63) + 64 ones-columns (64-127) per (key tile, head): the
        # PV matmul then emits the numerator on partitions 0-63 AND the
        # softmax denominator replicated across partitions 64-127 — the
        # partition broadcast of 1/den is never needed.
        vo = const.tile([128, NJ, 8, 128], BF16, tag="vo")

        # -------- DMA emission, ordered so compute can start eagerly -----
        # The HWDGE ring drains in FIFO order, so the order below IS the
        # arrival order.  The first Kproj chunk is gated only on wk's m0
        # block (0.25 MB) + the first 512-key xkv chunk (1 MB), so compute
        # starts ~5 us in instead of waiting for whole tensors.  K-side
        # operands ride the SP ring, Q-side the ACT ring (separate hardware
        # rings that drain concurrently).
        xt_r = xt_d[:, :].rearrange("(k p) c -> p k c", k=NK)
        xkv_r = xkv_d[:, :].rearrange("(k p) c -> p k c", k=NK)
        wq_r = wq_d[:, :].rearrange("(m k p) c -> p m k c", m=NM, k=NK)
        wk_r = wk_d[:, :].rearrange("(m k p) c -> p m k c", m=NM, k=NK)
        wv_r = wv_d[:, :].rearrange("(k p) c -> p k c", k=NK)
        # SP ring: first-scores gate, then V-side, then later head pairs
        nc.sync.dma_start(out=wk0, in_=wk_r[:, 0])
        for i, csl in enumerate(kv_slices):
            nc.sync.dma_start(out=xkvs[i], in_=xkv_r[:, :, csl])
        nc.sync.dma_start(out=wv, in_=wv_r)
        nc.sync.dma_start(out=wkr, in_=wk_r[:, 1:NM])
        # ACT ring: Q-side
        nc.scalar.dma_start(out=wq0, in_=wq_r[:, 0])
        nc.scalar.dma_start(out=xts[0], in_=xt_r[:, :, q_slices[0]])
        nc.scalar.dma_start(out=xts[1], in_=xt_r[:, :, q_slices[1]])
        nc.scalar.dma_start(out=wqr, in_=wq_r[:, 1:NM])
        nc.scalar.dma_start(out=xts[2], in_=xt_r[:, :, q_slices[2]])
        nc.scalar.dma_start(out=xts[3], in_=xt_r[:, :, q_slices[3]])
        ms_pool = nc.gpsimd.memset(vo, 1.0)  # ones cols; V copies fill 0-63
        # DVE pre-touch: observe the gpsimd memset tick once, so the per-tile
        # V copies don't each need a second (Pool) sync wait — the DVE ISA
        # struct has a single wait slot.
        nc.vector.memset(vo[0:1, 0, 0, 64:65], 1.0)
        # Pad-key kill: overwrite the last npt tiles' ones-columns with the
        # per-core 0/1 block (0 rows = pad keys).  Emitted LAST on the ACT
        # ring: its parked wait (on the vo memset) must not delay the
        # Q-side loads, and PV first reads these tiles ~15 ticks in.
        nc.scalar.dma_start(
            out=vo[:, NJ - npt:NJ, :, 64:128],
            in_=vo_d[:, :].rearrange("p (t h d) -> p t h d", t=npt, h=8))

        # ---- mode-group ordering hook: the scheduler dispatches by
        # simulated readiness (priority only tie-breaks among READY
        # instructions), so without a dep it hoists 128-row-mode PV/proj
        # matmuls between the two score ticks of a window, paying the
        # 64<->128 tiling-mode drain (~106 ns) twice per tick instead of
        # twice per window.  The first 128-mode matmul after each scores
        # section gets an explicit dep on the section's last score matmul.
        first_dep = {"ins": None}

        def tmm(*a, **kw):
            i = nc.tensor.matmul(*a, **kw)
            if first_dep["ins"] is not None:
                add_dep_helper(i.ins, first_dep["ins"],
                               reason="mode-group order")
                first_dep["ins"] = None
            return i

        # ---------------- QKV projection chunk emitters ----------------
        # Each chunk is ~8 accumulating matmuls + one DVE evict.  Chunks
        # are interleaved into the 128-row-mode sections below, and use
        # their OWN 1-bank PSUM ring so they never steal the scores ring's
        # lookahead slot (which would lockstep the PE with the ScalarE).
        def _proj_half(isq, x_t, dest, m, csl, half):
            w0, wr = (wq0, wqr) if isq else (wk0, wkr)
            ps = proj_open.get((isq, m, csl.start))
            if ps is None:
                ps = ppool.tile([128, 512], F32, tag="pj",
                                name=f"pj{m}_{csl.start}")
                proj_open[(isq, m, csl.start)] = ps
            pslice = ps[:, 0:csl.stop - csl.start]
            ks = range(NK // 2) if half == 0 else range(NK // 2, NK)
            for k in ks:
                tmm(
                    pslice,
                    lhsT=w0[:, k, :] if m == 0 else wr[:, m - 1, k, :],
                    rhs=x_t[:, k, 0:csl.stop - csl.start],
                    start=(k == 0),
                    stop=(k == NK - 1),
                )
            if half == 1:
                del proj_open[(isq, m, csl.start)]
                nc.vector.tensor_copy(dest[:, m, csl], pslice)

        proj_open = {}

        def kproj_chunk(m, ci, half=None):
            csl = kv_slices[ci]
            if half is None:
                _proj_half(False, xkvs[ci], kt, m, csl, 0)
                _proj_half(False, xkvs[ci], kt, m, csl, 1)
            else:
                _proj_half(False, xkvs[ci], kt, m, csl, half)

        def qproj_chunk(m, ci, half=None):
            csl = q_slices[ci]
            if half is None:
                _proj_half(True, xts[ci], qt, m, csl, 0)
                _proj_half(True, xts[ci], qt, m, csl, 1)
            else:
                _proj_half(True, xts[ci], qt, m, csl, half)

        def vproj_chunk(st):
            ps = ppool.tile([128, 512], F32, tag="pj", name=f"pv{st}")
            pslice = ps[:, 0:512]
            for k in range(NK):
                tmm(
                    pslice,
                    lhsT=xkvs[st // 4][:, k, (st % 4) * 128:
                                       (st % 4 + 1) * 128],
                    rhs=wv[:, k, :],
                    start=(k == 0),
                    stop=(k == NK - 1),
                )
            nc.vector.tensor_copy(
                vo[:, st, :, 0:64],
                pslice.rearrange("p (h d) -> p h d", h=8),
            )

        # ------- attention: software-pipelined global instruction stream --
        # Engines execute their instruction streams strictly in order, so a
        # single instruction parked on an unmet semaphore stalls everything
        # traced after it on that engine.  To keep both the ScalarE exp
        # stream and the PE dense, the trace is emitted as one global
        # pipeline in two-tick windows: scores/exp for ticks (g, g+1) in
        # 64-row tiling mode, then PV matmuls for ticks (g-LAG, g-LAG+1)
        # plus projection chunks in 128-row mode, each iteration's
        # softmax-normalization tail staggered a few ticks after its last
        # PV.
        LAG = 8
        NIT = NM * NQC          # 16 (pair, qchunk) iterations
        NG = NIT * NJ           # scores/exp ticks
        tail_deps = []
        lasts = {}
        ex_ring = {}            # tick -> exp tile
        pv_of = {}              # it -> [pva, pvb]
        cstate = {}             # (it, hh) -> dict with combine intermediates
        pending = {}            # tick -> list of closures (combine tails)
        proj_at = {}            # tick -> list of proj-chunk closures
        exp_of_it = {}          # it -> first exp instruction (NOP anchors)

        def at(g, fn):
            proj_at.setdefault(g, []).append(fn)

        # PV emission ticks: each iteration's first PV is deferred one
        # tick past the boundary so its parked wait (on the previous
        # iteration's PSUM evict) cannot block the boundary scores.
        pv_at = {}
        for h in range(NG):
            defer = 1 if h % NJ == 0 else 0
            pv_at.setdefault(h + LAG + defer, []).append(h)

        # V j-tiles 4+: vo[v] consumed by PV at tick v+LAG; tiles 0-3 run
        # in the prelude, the rest early in window 0.
        for v in range(4, NJ):
            at(min(2 * (v - 4), v + LAG - 3), lambda v=v: vproj_chunk(v))
        # Q^T m=0 tail chunks: qt[m0, csl c] consumed at tick NJ*c.
        for c in range(1, NQC):
            at(NJ * c - 7, lambda c=c: qproj_chunk(0, c))
        # Pair p>=1: K^T m-tile chunk i covers key tiles ~[4i, 4i+4) of the
        # pair's scores (consumed from tick WIN*p + 4i); Q chunk c consumed
        # from tick WIN*p + NJ*c.
        for p in range(1, NM):
            for i in range(len(kv_slices)):
                t0 = WIN * p - 5 if i == 0 else WIN * p + 4 * i - 4
                at(t0, lambda p=p, i=i: kproj_chunk(p, i, 0))
                at(t0 + 1, lambda p=p, i=i: kproj_chunk(p, i, 1))
            at(WIN * p - 9, lambda p=p: qproj_chunk(p, 0, 0))
            at(WIN * p - 8, lambda p=p: qproj_chunk(p, 0, 1))
            for c in range(1, NQC):
                at(WIN * p + NJ * c - 6,
                   lambda p=p, c=c: qproj_chunk(p, c, 0))
                at(WIN * p + NJ * c - 5,
                   lambda p=p, c=c: qproj_chunk(p, c, 1))

        def emit_scores(g):
            it, j = divmod(g, NJ)
            p, q = divmod(it, NQC)
            qsl = slice(q * 512, (q + 1) * 512)
            jsl = slice(j * 128, (j + 1) * 128)
            ps = spool.tile([128, 1024], F32, tag="sc", name=f"ps{g}")
            # scores^T: head A on array half T0 (partitions 0-63), head B
            # on T8 — the two 64-row-mode matmuls run CONCURRENTLY.
            nc.tensor.matmul(
                ps[:, 0:512], lhsT=kt[0:64, p, jsl],
                rhs=qt[0:64, p, qsl], start=True, stop=True)
            lasts["smm"] = nc.tensor.matmul(
                ps[:, 512:1024], lhsT=kt[64:128, p, jsl],
                rhs=qt[64:128, p, qsl], start=True, stop=True)
            # exp over both heads' scores in one ScalarE pass
            ex = expool.tile([128, 1024], BF16, tag="ex", name=f"ex{g}")
            lasts["exp"] = nc.scalar.activation(ex, ps, EXPFN)
            ex_ring[g] = ex

        def emit_pv(g):
            it, j = divmod(g, NJ)
            p, q = divmod(it, NQC)
            if j == 0:
                pv_of[it] = [
                    pvpool.tile([128, 512], F32, tag=t, name=f"{t}_{it}")
                    for t in ("pva", "pvb")]
            pva, pvb = pv_of[it]
            ex = ex_ring.pop(g)
            kw = dict(start=(j == 0), stop=(j == NJ - 1))
            tmm(pva, lhsT=vo[:, j, 2 * p, :],
                rhs=ex[:, 0:512], **kw)
            tmm(pvb, lhsT=vo[:, j, 2 * p + 1, :],
                rhs=ex[:, 512:1024], **kw)
            if j == NJ - 1:
                # The PSUM-slot-freeing evict copies run INLINE right after
                # the stop matmuls (the next iteration's PV matmuls wait on
                # these slot releases); the rest of the chain (gather ->
                # recip -> bounce -> replicate -> mul -> store) is spread
                # over the following ticks — latency, not engine pressure.
                combine1(it, 0)
                combine1(it, 1)
                for hh in (0, 1):
                    pending.setdefault(g + 1 + hh, []).append(
                        lambda it=it, hh=hh: combine2(it, hh))
                    pending.setdefault(g + 3 + hh, []).append(
                        lambda it=it, hh=hh: combine2b(it, hh))
                    pending.setdefault(g + 5 + hh, []).append(
                        lambda it=it, hh=hh: combine3(it, hh))

        def combine1(it, hh):
            """Evict one head's PV tile (numerator rows 0-63, replicated
            denominator rows 64-127) PSUM -> SBUF; frees its pv bank."""
            pv = pv_of[it][hh]
            s = cstate[(it, hh)] = {}
            ssum = comb.tile([128, 512], F32, tag=f"ssum{hh}",
                             name=f"ssum{it}_{hh}")
            # DVE pre-touch: the WAR wait on the previous generation's
            # gpsimd multiply lands here, leaving the evict copy with only
            # its PV-producer wait (1-wait ISA struct).
            nc.vector.memset(ssum[0:1, 0:1], 0.0)
            s["add"] = nc.vector.tensor_copy(ssum, pv)
            s["ssum"] = ssum

        def combine2(it, hh):
            """Gather the 512 denominators (row 64 of ssum, replicated) into
            a [128, 4] tile: the iterative-divide DVE reciprocal costs
            ~6.5 ns per element PER LANE, so the [64, 512] layout would be
            3.3 us while [128, 4] is ~0.2 us for the same 512 values."""
            s = cstate[(it, hh)]
            # The LAST iteration's chain DMAs ride the ACT ring: after the
            # final exp that ring is idle, so its parked waits cannot block
            # anything — and the SP ring's tail (other iterations' chains
            # and stores) drains concurrently instead of serializing.
            eng = nc.scalar if it == NIT - 1 else nc.sync
            # the last iteration gets fresh tiles (no ring WAR waits — the
            # ACT ring has no free slots to park excess waits on)
            pool = lastp if it == NIT - 1 else comb
            dT = pool.tile([128, 4], F32, tag=f"dT{hh if it == NIT - 1 else ''}",
                           name=f"dT{it}_{hh}")
            # NOP dep'd on the producer absorbs the DVE wait into the ring
            # engine's observed clock (1-wait DMA ISA struct)
            nop_i = eng.nop(nofuse=True, hint=f"gaw{it}_{hh}")
            add_dep_helper(nop_i.ins, s["add"].ins, reason="gather wait")
            s["gather"] = eng.dma_start(out=dT, in_=s["ssum"][64:65, :])
            s["dT"] = dT

        def combine2b(it, hh):
            """Cheap reciprocal, then bounce the 512 values through a DRAM
            row and read them back replicated across partitions 0-63 with a
            stride-0 DRAM source AP."""
            s = cstate[(it, hh)]
            eng = nc.scalar if it == NIT - 1 else nc.sync
            u = 2 * it + hh
            pool = lastp if it == NIT - 1 else comb
            recT = pool.tile([128, 4], F32,
                             tag=f"recT{hh if it == NIT - 1 else ''}",
                             name=f"recT{it}_{hh}")
            # DVE pre-touch: the WAR wait on the previous generation's
            # scatter-DMA lands here, so the reciprocal carries only the
            # gather-completion wait (1-wait ISA struct).
            nc.vector.memset(recT[0:1, 0:1], 0.0)
            rc_i = nc.vector.reciprocal(recT, s["dT"])
            nop_i = eng.nop(nofuse=True, hint=f"scw{it}_{hh}")
            add_dep_helper(nop_i.ins, rc_i.ins, reason="scatter wait carry")
            s["scat"] = eng.dma_start(out=dbn_d[u:u + 1, :], in_=recT)
            rec = pool.tile([64, 512], F32,
                            tag=f"rec{hh if it == NIT - 1 else ''}",
                            name=f"rec{it}_{hh}")
            src = dbn_d[u:u + 1, :]
            bsrc = bass.AP(src.tensor, src.offset,
                           [[0, 64]] + list(src.ap)[1:])
            nop2 = eng.nop(nofuse=True, hint=f"rpw{it}_{hh}")
            add_dep_helper(nop2.ins, s["scat"].ins, reason="replicate wait")
            s["dma"] = eng.dma_start(out=rec, in_=bsrc)
            s["rec"] = rec

        def combine3(it, hh):
            """Final multiply on the otherwise-idle GpSimd engine (all
            operands are SBUF) — keeps the rec-DMA completion wait and the
            multiply itself off the busy DVE stream."""
            p, q = divmod(it, NQC)
            s = cstate.pop((it, hh))
            eng = nc.scalar if it == NIT - 1 else nc.sync
            ot = outp.tile([64, 512], F32, tag="ot", name=f"ot{it}_{hh}")
            # absorb the rec-DMA and ssum-producer waits into the GpSimd
            # observed clock (1-wait engine ISA structs)
            scr = comb.tile([1, 1], F32, tag="scr", name=f"scr{it}_{hh}")
            m1 = nc.gpsimd.memset(scr, 0.0)
            add_dep_helper(m1.ins, s["dma"].ins, reason="rec wait carry")
            m2 = nc.gpsimd.memset(scr, 0.0)
            add_dep_helper(m2.ins, s["add"].ins, reason="ssum wait carry")
            nc.gpsimd.memset(ot[0:1, 0:1], 0.0)
            lasts["mul"] = nc.gpsimd.tensor_mul(
                ot, s["ssum"][0:64, :], s["rec"])
            nop_i = eng.nop(nofuse=True, hint=f"stw{it}_{hh}")
            add_dep_helper(nop_i.ins, lasts["mul"].ins,
                           reason="store wait carry")
            row0 = q * DCOL + p * 128 + hh * 64
            st_i = eng.dma_start(out=out_d[row0:row0 + 64, :], in_=ot)
            tail_deps.append(st_i)

        # ---------------- prelude: work for the first ticks ----------------
        for ci in range(len(kv_slices)):
            kproj_chunk(0, ci)
        qproj_chunk(0, 0)
        for v in range(4):
            vproj_chunk(v)

        for w in range(0, NG + LAG + 12, 2):
            win = (w, w + 1)
            for g in win:
                for fn in pending.pop(g, ()):
                    fn()
            # ---- 64-row-mode section: scores (+ their exps) ----
            emitted_scores = False
            for g in win:
                if g < NG:
                    emit_scores(g)
                    emitted_scores = True
                    if g % NJ == 0:
                        exp_of_it[g // NJ] = lasts["exp"]
            if emitted_scores:
                # next 128-mode matmul must follow this window's scores
                first_dep["ins"] = lasts["smm"].ins
            if w % 4 == 0:
                # Spread zero-wait SP slots through the stream for the
                # wait legalizer.  The dep is only for PLACEMENT: use a
                # long-completed instruction (two iterations back) so
                # the NOP's wait never stalls the SP stream — a wait on
                # the current exp would hold up every store/DMA queued
                # behind it (convoy through the ot-slot WAR).
                anchor = exp_of_it.get(min(w // NJ, NIT - 1) - 1, ms_pool)
                for k in range(8):
                    nop_i = nc.sync.nop(nofuse=True, hint=f"pad{w}_{k}")
                    add_dep_helper(nop_i.ins, anchor.ins,
                                   reason="legalizer slot padding")
            # ---- 128-row-mode section: PVs, then projection chunks ----
            for g in win:
                for h in pv_at.pop(g, ()):
                    emit_pv(h)
            for g in win:
                for fn in proj_at.pop(g, ()):
                    fn()
        for g in sorted(pending):
            for fn in pending[g]:
                fn()
        pending.clear()
        for g in sorted(proj_at):
            for fn in proj_at[g]:
                fn()
        proj_at.clear()
        # Trailing SP no-ops, each depending on one late instruction: the
        # SP sequencer then observes every proc's final semaphore tick
        # before the kernel-tail Drain, whose ISA struct takes only a
        # single sync wait, so Tile elides the rest.
        last_store = tail_deps[-1]
        tail_deps += [lasts["exp"], lasts["mul"], ms_pool]
        for d in tail_deps:
            nop_i = nc.sync.nop(nofuse=True, hint="tailpad")
            add_dep_helper(nop_i.ins, d.ins,
                           reason="spread tail drain waits")
        for _ in range(10):  # zero-wait late slots for the legalizer
            nop_i = nc.sync.nop(nofuse=True, hint="tailpad2")
            add_dep_helper(nop_i.ins, last_store.ins,
                           reason="late zero-wait slot")
    _spread_matmul_waits(nc)
    return nc


def _spread_matmul_waits(nc):
    """The walrus in this container accepts only ONE sync-wait command per
    compute-engine ISA struct (Matmult/Activation/TensorCopy/...), but the
    Tile scheduler sometimes attaches two.  Fix: move excess waits onto an
    earlier instruction of the same engine (which executes first, so the
    ordering the wait enforces is preserved).

    Safety: a wait (sem, v) may move to predecessor p only if the
    instruction whose update makes sem reach v is scheduled BEFORE p.
    That keeps every wait's producer strictly earlier in the schedule, so
    the event order stays acyclic (no introduced deadlocks)."""
    import bass_rust

    SKIP_OPCODES = {"EventSemaphore"}
    if True:
        insts = [i for blk in nc.m.functions[0].blocks
                 for i in blk.instructions]
        # cumulative sem counts in schedule order -> producer position
        sem_hist = {}   # sem id -> list of (position, cumulative_value)
        for pos, inst in enumerate(insts):
            si = inst.sync_info
            if si is None:
                continue
            for u in si.on_update:
                hist = sem_hist.setdefault(u.id, [])
                prev = hist[-1][1] if hist else 0
                hist.append((pos, prev + (u.update_value or 1)))

        def producer_pos(w):
            for pos, cum in sem_hist.get(w.id, ()):
                if cum >= w.wait_value:
                    return pos
            return None  # produced outside this block (host/runtime)

        def exec_unit(inst):
            """Sequential dispatch domain: the issuing engine sequencer.
            DMACopy waits are polled by the issuing sequencer (SP/ACT)
            before the descriptor is pushed, so they move within that
            engine's stream like any other instruction's waits."""
            return str(getattr(inst, "engine", None))

        # which execution units increment each semaphore.  DMA-completion
        # semaphores (DMAHW*/DMASW*) increment asynchronously at transfer
        # completion, NOT at dispatch — never treat them as same-engine.
        sem_engines = {}
        for pos, inst in enumerate(insts):
            si = inst.sync_info
            if si is None:
                continue
            for u in si.on_update:
                if u.ant_name.startswith(("DMAHW", "DMASW")):
                    sem_engines.setdefault(u.id, set()).add("ASYNC_DMA")
                else:
                    sem_engines.setdefault(u.id, set()).add(exec_unit(inst))

        n_waits = [len(i.sync_info.on_wait) if i.sync_info else 0
                   for i in insts]
        # positions of instructions per execution unit, in order
        eng_of = [exec_unit(i) for i in insts]
        # per-engine observed semaphore clock: once an engine's stream has
        # waited for (sem >= v), every later instruction on that stream
        # observes it — later waits with value <= v are redundant.
        obs = {}

        def observed(eng, w):
            return obs.get((eng, w.id), -1) >= w.wait_value

        def observe(eng, w):
            key = (eng, w.id)
            if obs.get(key, -1) < w.wait_value:
                obs[key] = w.wait_value

        for pos, inst in enumerate(insts):
            eng = eng_of[pos]
            if inst.opcode in SKIP_OPCODES or \
                    not eng.startswith("EngineType."):
                if inst.sync_info:
                    for w in inst.sync_info.on_wait:
                        observe(eng, w)
                continue
            si = inst.sync_info
            if si is None:
                continue
            waits = list(si.on_wait)
            if waits:
                # drop waits already covered by this engine's stream
                waits = [w for w in waits if not observed(eng, w)]
                # Engines retire instructions strictly in order (PE MMs are
                # pc-monotone in start AND end even across array tiles), so
                # a wait on a semaphore only ever incremented synchronously
                # by THIS engine's earlier instructions is trivially
                # satisfied: drop.  (Async DMA-completion sems excluded.)
                waits = [w for w in waits
                         if sem_engines.get(w.id) != {eng}]
            # Custom-DVE InstISA structs cannot encode ANY sync wait in
            # this walrus ("ISA wrong length"); all their waits must move
            # to earlier same-engine slots.
            max_w = 0 if inst.opcode == "ISA" else 1
            if len(waits) > max_w:
                # keep one wait in place, move the rest to earlier free
                # slots on the same engine stream (after each wait's
                # producer, so the event order stays acyclic).  Prefer
                # keeping the latest-produced wait; fall back to other
                # keep choices if the excess can't be placed.
                waits.sort(key=lambda w: producer_pos(w) or len(insts))

                def try_place(keep_idx):
                    placement, used = [], set()
                    for wi, w in enumerate(waits):
                        if keep_idx is not None and wi == keep_idx:
                            continue
                        pp = producer_pos(w)
                        if pp is None:
                            return None
                        tgt = None
                        for q in range(pos - 1, pp, -1):
                            if eng_of[q] == eng and n_waits[q] == 0 and \
                                    q not in used and \
                                    insts[q].opcode not in SKIP_OPCODES:
                                tgt = q
                                break
                        if tgt is None:
                            return None
                        used.add(tgt)
                        placement.append((w, tgt))
                    return placement

                placement = None
                keep_choices = ([None] if max_w == 0 else
                                list(range(len(waits) - 1, -1, -1)))
                for keep_idx in keep_choices:
                    placement = try_place(keep_idx)
                    if placement is not None:
                        keep = None if keep_idx is None else waits[keep_idx]
                        break
                assert placement is not None, \
                    f"{inst.name}: cannot place excess waits " \
                    f"{[(w.ant_name, w.wait_value) for w in waits]}"
                for w, tgt in placement:
                    ti = insts[tgt]
                    tsi = ti.sync_info
                    ti.sync_info = bass_rust.SyncInfo(
                        on_wait=[w],
                        on_update=list(tsi.on_update)
                        if tsi is not None else [],
                    )
                    n_waits[tgt] = 1
                    observe(eng, w)
                waits = [keep] if keep is not None else []
            si.on_wait = waits
            inst.sync_info = si
            n_waits[pos] = len(waits)
            for w in waits:
                observe(eng, w)


def _prep_inputs(inputs, attention_mask, Wq, bq, Wk, bk, Wv, bv):
    """Host-side shard + layout prep.  Masked-out keys (exactly-0 softmax
    weight in the reference: exp(-10000-ish) underflows) are compacted away
    from the K/V sequence axis; pad positions are killed by zeroing their
    ones-columns (the `vones` 0/1 block).
    Returns (per-core input maps, nk, skv, npt)."""
    bf16 = ml_dtypes.bfloat16
    scale = 1.0 / np.sqrt(np.float32(DH))
    masks = np.asarray(attention_mask)
    has_bias = any(
        np.any(np.asarray(bias, np.float32) != 0) for bias in (bq, bk, bv))
    nk = 9 if has_bias else 8
    kpad = nk * 128
    counts = [int(masks[b].sum()) for b in range(B)]
    skv = ((max(counts) + 127) // 128) * 128
    nj = skv // 128
    # number of tail key tiles that may contain pad keys (uniform across
    # cores — the program is shared)
    npt = max(1, min(nj, (skv - min(counts) + 127) // 128))

    in_maps = []
    xcache = {}
    for c in range(NCORES):
        b, hg = c // 2, c % 2
        if b not in xcache:
            xtf = np.asarray(inputs[b], dtype=np.float32).T  # [D, S]
            xt = np.zeros((kpad, S), dtype=bf16)
            xt[0:D, :] = xtf.astype(bf16)
            idx = np.nonzero(masks[b])[0]
            cnt = len(idx)
            xkv = np.zeros((kpad, skv), dtype=bf16)
            xkv[0:D, 0:cnt] = xtf[:, idx].astype(bf16)
            if has_bias:
                xt[D, :] = bf16(1.0)
                xkv[D, :] = bf16(1.0)
            # 0/1 ones-column block for the last npt key tiles
            vones = np.zeros((128, npt, 8, 64), dtype=bf16)
            for t in range(npt):
                base = (nj - npt + t) * 128
                valid = max(0, min(128, cnt - base))
                vones[0:valid, t, :, :] = bf16(1.0)
            xcache[b] = (xt, xkv, vones.reshape(128, npt * 512))
        xt, xkv, vones = xcache[b]
        cols = slice(hg * DCOL, (hg + 1) * DCOL)

        def wpack(W, bias, s=np.float32(1.0)):
            w = np.zeros((kpad, DCOL), dtype=bf16)
            w[0:D, :] = (np.asarray(W, np.float32)[:, cols] * s).astype(bf16)
            if has_bias:
                w[D, :] = (np.asarray(bias, np.float32)[cols] * s
                           ).astype(bf16)
            return w

        def mmajor(w):  # [kpad, DCOL] -> [(m k p), 128]
            return np.ascontiguousarray(
                w.reshape(nk, 128, NM, 128).transpose(2, 0, 1, 3)
                .reshape(NM * kpad, 128))

        in_maps.append({
            "xt": xt,
            "xkv": xkv,
            "wq": mmajor(wpack(Wq, bq)),
            "wk": mmajor(wpack(Wk, bk, scale)),
            "wv": wpack(Wv, bv),
            "vones": vones,
        })
    return in_maps, nk, skv, npt


_NC_CACHE = {}


def _get_nc(nk, skv, npt):
    key = (nk, skv, npt)
    if key not in _NC_CACHE:
        _NC_CACHE[key] = build_nc(nk, skv, npt)
    return _NC_CACHE[key]


def _assemble(results):
    full = np.empty((B, S, D), dtype=np.float32)
    for c in range(NCORES):
        b, hg = c // 2, c % 2
        arr = np.asarray(results[c]["out"], dtype=np.float32)
        # [NQC*DCOL, 512] q-chunk-major -> [S, DCOL]
        full[b, :, hg * DCOL:(hg + 1) * DCOL] = \
            arr.reshape(NQC, DCOL, 512).transpose(0, 2, 1).reshape(S, DCOL)
    return full


def _ensure_ntff_hook():
    """Inject the missing antenv.axon_hooks module so trace=True works."""
    import types
    try:
        from antenv import axon_hooks  # noqa: F401
        return
    except ImportError:
        pass
    import antenv
    mod = types.ModuleType("antenv.axon_hooks")
    mod._hook = None

    def set_axon_ntff_profile_hook(h):
        mod._hook = h

    def get_axon_ntff_profile_hook():
        return mod._hook

    mod.set_axon_ntff_profile_hook = set_axon_ntff_profile_hook
    mod.get_axon_ntff_profile_hook = get_axon_ntff_profile_hook
    sys.modules["antenv.axon_hooks"] = mod
    antenv.axon_hooks = mod
    from trn_agent_boot.trn_boot import _ntff_profile_via_ctypes
    mod.set_axon_ntff_profile_hook(
        _ntff_profile_via_ctypes("/opt/axon/libaxon_pjrt.so"))


def run(trace=False, **inputs):
    """Run on hardware; returns (output, BassKernelResults)."""
    from concourse.bass_utils import run_bass_kernel_spmd
    if trace:
        _ensure_ntff_hook()
    in_maps, nk, skv, npt = _prep_inputs(**inputs)
    nc = _get_nc(nk, skv, npt)
    res = run_bass_kernel_spmd(
        nc, in_maps, core_ids=list(range(NCORES)), trace=trace)
    return _assemble(res.results), res


def kernel(**inputs):
    out, _ = run(trace=False, **inputs)
    return out
